# revision 38
# baseline (speedup 1.0000x reference)
# Laplacian normalization kernel for Trainium2 (8 NeuronCores, SPMD).
#
# out = d^-1/2[:, None] * A * d^-1/2[None, :],  d_i = sum_j A[i, j],  A: [8192, 8192] f32
#
# The correctness gate is rel_err < 2e-2, so the whole data path runs in
# bf16 (~1.3% end-to-end max rel err measured on the real inputs): the host
# casts A to bf16, the device reads/writes bf16, the host upcasts the
# result. That halves HBM traffic vs f32 AND lets the full 16MB row-shard
# stay resident in SBUF (128KB of ~208KB/partition), so A is read once.
#
# Per core (1024 rows = 8 tiles of [128, 8192] bf16):
#   phase A: 16 x 1MB loads alternating both HWDGE queues (~52us). Row
#   sums ride along on ACT (activation-Copy with accum_out, ~7us/tile)
#   and DVE (reduce_sum, 1x, ~9us/tile) — DVE alone would take 74us since
#   tensor_reduce has no fast mode. The last TWO tiles are split
#   half-and-half across both engines so the final row-sum lands ~4us
#   after the last load.
#   collective: d^-1/2 f32 -> PE-transpose -> bf16 [1,1024] -> AllGather
#   (2KB -> 16KB). A warmup AllGather at t=0 absorbs cc-stream setup.
#   During the collective window DVE row-scales all tiles in place
#   (tensor_scalar, 4x mode, ~19us — needs no remote data).
#   phase C: cvec ([128,8192] bf16 = gathered d^-1/2 replicated across
#   partitions) is built by a PE ones-matmul into PSUM + ACT copy-downcast
#   per 2048-col chunk instead of a 2MB broadcast DMA: keeps the
#   partition-broadcast OFF the DMA queues, which the 16MB of stores
#   saturate. Then per half-tile: tensor_tensor by cvec (2x mode) + store,
#   alternating queues.

import numpy as np

N = 8192
NCORES = 8
R = N // NCORES  # 1024 rows per core
P = 128          # SBUF partitions
T = R // P       # 8 row-tiles of [128, 8192] per core
HC = 2           # load/store chunks per tile (1MB each)
W = N // HC      # chunk width (4096 columns)
BC = 16          # cvec build chunks
BW = N // BC     # 512 columns: matmul out must fit one 2KB PSUM bank

ACT_TILES = (0, 2, 4, 6)  # full-tile row-sums on ACT; DVE takes 1, 3, 5
SPLIT_TILES = (7,)        # split ACT/DVE half-and-half (critical tail)

_cache = {}


def _build():
    import concourse.bacc as bacc
    import concourse.mybir as mybir
    import concourse.tile as tile
    from concourse import masks

    f32 = mybir.dt.float32
    bf16 = mybir.dt.bfloat16
    X = mybir.AxisListType.X
    mult = mybir.AluOpType.mult
    add = mybir.AluOpType.add
    Copy = mybir.ActivationFunctionType.Copy

    nc = bacc.Bacc(
        "TRN2", target_bir_lowering=False, debug=False, num_devices=NCORES
    )
    a = nc.dram_tensor("a_shard", [R, N], bf16, kind="ExternalInput").ap()
    out = nc.dram_tensor("out_shard", [R, N], bf16, kind="ExternalOutput").ap()

    a_t = a.rearrange("(t p) n -> t p n", p=P)
    o_t = out.rearrange("(t p) n -> t p n", p=P)

    with tile.TileContext(nc) as tc:
        with (
            tc.tile_pool(name="cpool", bufs=1) as cpool,
            tc.tile_pool(name="vpool", bufs=1) as vpool,
            tc.tile_pool(name="psum", bufs=1, space="PSUM") as psum,
            tc.tile_pool(name="bpsum", bufs=3, space="PSUM") as bpsum,
            tc.tile_pool(name="dram", bufs=1, space="DRAM") as dram,
        ):
            dsum = vpool.tile([P, T], f32, tag="dsum")
            dinv = vpool.tile([P, T], f32, tag="dinv")
            hp = vpool.tile([P, 12], f32, tag="hp")
            cvec = vpool.tile([P, N], bf16, tag="cvec")
            ident = vpool.tile([P, P], f32, tag="ident")
            ones = vpool.tile([8, P], bf16, tag="ones")
            dvec = vpool.tile([8, N], bf16, tag="dvec")
            dinv_tp = vpool.tile([T, P], bf16, tag="dinv_tp")
            dinv_tpp = psum.tile([T, P], f32, tag="dinv_tpp")
            dloc = dram.tile([1, R], bf16, tag="dloc")
            dfull = dram.tile([1, N], bf16, tag="dfull")
            warm = dram.tile([1, 8], bf16, tag="warm")
            warm_o = dram.tile([1, 8 * NCORES], bf16, tag="warm_o")

            # 3MB of loads ride the SWDGE (Pool) queue as a third DMA path
            # — dispatched before the warmup collective so the in-order
            # Pool queue cannot block them. Targets are DVE-reduced tiles,
            # which have end-of-phase slack if SWDGE turns out slower.
            tiles = []
            for t in range(T):
                big = cpool.tile([P, N], bf16, tag=f"c{t}")
                tiles.append(big)
            GP_LOADS = ((3, 0), (3, 1), (5, 0))
            for t, h in GP_LOADS:
                cols = slice(h * W, (h + 1) * W)
                nc.gpsimd.dma_start(
                    out=tiles[t][:, cols], in_=a_t[t][:, cols]
                )

            # warmup collective: absorbs cc-stream setup before the timed
            # dependency chain needs it
            nc.vector.memset(dinv_tp[0:1, 0:8], 0.0)
            nc.gpsimd.dma_start(out=warm[0, :], in_=dinv_tp[0:1, 0:8])
            nc.gpsimd.collective_compute(
                "AllGather",
                mybir.AluOpType.bypass,
                replica_groups=[list(range(NCORES))],
                ins=[warm[0, :].opt()],
                outs=[warm_o[0, :].opt()],
            )

            masks.make_identity(nc, ident[:, :])
            nc.vector.memset(ones[:, :], 1.0)

            ld = [nc.sync, nc.scalar]
            nld = [0]

            def hw_loads(t):
                if t == 3:
                    return  # fully on the Pool queue
                big = tiles[t]
                # middle tiles load as ONE 2MB DMA (16KB/partition lines,
                # half the descriptors) for better sustained bandwidth;
                # tiles 0/1/7 stay chunked (pipe fill / critical tail)
                nch = 4 if t in (0, 1, T - 1) else (2 if t == 5 else 1)
                w = N // nch
                for h in range(nch):
                    if (t, h * w // W) in GP_LOADS and w == W:
                        continue  # already dispatched on the Pool queue
                    cols = slice(h * w, (h + 1) * w)
                    ld[nld[0] % 2].dma_start(
                        out=big[:, cols], in_=a_t[t][:, cols]
                    )
                    nld[0] += 1

            # phase A: load dispatches are software-pipelined TWO tiles
            # ahead of the reduces. nc.scalar.dma_start is dispatched by
            # the ACT engine, and the HWDGE ring holds only ~4
            # descriptors: dispatching everything up front blocks the
            # engine on ring space (measured +25us), while dispatching
            # each tile right before its own reduce starves the ring
            # behind every 7us ACT reduce (measured ~12us/tile of lag).
            # Two tiles of lookahead keeps the ring full either way.
            # Tiles 0/1 load in 2048-col chunks to fill the DMA pipe
            # faster at startup; tile 7's second half loads and reduces in
            # 2048-col quarters so the last (critical-path) reduce is
            # short.
            hw_loads(0)
            hw_loads(1)
            for t in range(T):
                big = tiles[t]
                w = N // 4
                if t + 2 < T:
                    hw_loads(t + 2)
                if t in SPLIT_TILES:
                    # ACT takes the first half; DVE reduces the last two
                    # quarters as they land, then combines
                    nc.scalar.activation(
                        out=big[:, 0:W],
                        in_=big[:, 0:W],
                        func=Copy,
                        accum_out=hp[:, 9:10],
                    )
                    nc.vector.reduce_sum(
                        out=hp[:, 10:11], in_=big[:, W : W + w], axis=X
                    )
                    nc.vector.reduce_sum(
                        out=hp[:, 11:12], in_=big[:, W + w : N], axis=X
                    )
                    nc.vector.tensor_scalar(
                        out=dsum[:, t : t + 1],
                        in0=hp[:, 10:11],
                        scalar1=hp[:, 9:10],
                        scalar2=hp[:, 11:12],
                        op0=add,
                        op1=add,
                    )
                elif t in ACT_TILES:
                    nc.scalar.activation(
                        out=big[:, :],
                        in_=big[:, :],
                        func=Copy,
                        accum_out=dsum[:, t : t + 1],
                    )
                else:
                    # DVE reduces go per half-tile: if the Tile scheduler
                    # misorders DVE ops against actual DMA arrival (seen on
                    # HW), the inversion costs <=4.4us instead of 8.7us
                    nc.vector.reduce_sum(
                        out=hp[:, t : t + 1], in_=big[:, 0:W], axis=X
                    )
                    nc.vector.reduce_sum(
                        out=hp[:, t + 1 : t + 2], in_=big[:, W:N], axis=X
                    )
                    nc.vector.tensor_tensor(
                        out=dsum[:, t : t + 1],
                        in0=hp[:, t : t + 1],
                        in1=hp[:, t + 1 : t + 2],
                        op=add,
                    )

            # d^-1/2 (ACT sqrt + DVE reciprocal; ACT Rsqrt is banned), then
            # PE-transpose [128, T] -> [T, P] so the collective input is one
            # contiguous 2KB DMA
            nc.scalar.sqrt(dsum[:, :], dsum[:, :])
            nc.vector.reciprocal(dinv[:, :], dsum[:, :])
            nc.tensor.transpose(dinv_tpp[:, :], dinv[:, :], ident[:, :])
            # PSUM->SBUF downcast on DVE: keeps ACT's second Copy-table
            # switch off the critical chain
            nc.vector.tensor_copy(out=dinv_tp[:, :], in_=dinv_tpp[:, :])
            nc.sync.dma_start(out=dloc[0, :], in_=dinv_tp[:, :])

            nc.gpsimd.collective_compute(
                "AllGather",
                mybir.AluOpType.bypass,
                replica_groups=[list(range(NCORES))],
                ins=[dloc[0, :].opt()],
                outs=[dfull[0, :].opt()],
            )

            # row scale (local, DVE 4x) — runs during the collective window
            for t in range(T):
                nc.vector.tensor_scalar(
                    out=tiles[t][:, :],
                    in0=tiles[t][:, :],
                    scalar1=dinv[:, t : t + 1],
                    scalar2=None,
                    op0=mult,
                )

            # cvec = gathered d^-1/2 replicated across partitions. The first
            # 1024 cols come from a small direct broadcast DMA (256KB) so
            # the first col-scale starts ~4us after the collective; the
            # rest is built on PE (ones[8,128].T @ dvec[8,N] = 8*d^-1/2 ->
            # PSUM) + ACT copy with scale=1/8 (exact exponent shift), which
            # keeps the bulk partition-broadcast off the DMA queues that
            # the stores saturate.
            nc.scalar.dma_start(
                out=cvec[:, 0 : 2 * BW],
                in_=dfull[0:1, 0 : 2 * BW].to_broadcast((P, 2 * BW)),
            )
            nc.sync.dma_start(
                out=dvec[:, :], in_=dfull[0:1, :].to_broadcast((8, N))
            )
            for b in range(2, BC):
                cols = slice(b * BW, (b + 1) * BW)
                pb = bpsum.tile([P, BW], f32, tag="bc")
                nc.tensor.matmul(
                    pb[:, :], ones[:, :], dvec[:, cols], start=True, stop=True
                )
                nc.scalar.activation(
                    out=cvec[:, cols], in_=pb[:, :], func=Copy, scale=0.125
                )

            # phase C: col scale (tensor_tensor, 2x) + store per half-tile.
            # Tile 0's first half goes in 1024-col slivers so the first
            # store launches as soon as cvec chunk 0 lands — the stores
            # are the phase-C bottleneck (~317 GB/s), every us of earlier
            # start is a us off the tail.
            # tile 0 in slivers (earliest stores), tile 1 in halves (one
            # rides SWDGE, capacity-capped at 2MB total), tiles 2-7 as
            # single full-row 2MB stores (16KB/partition lines, half the
            # descriptors, for better sustained write bandwidth)
            plan = [(0, b * BW * 2, (b + 1) * BW * 2) for b in range(4)]
            plan += [(0, W, N)]
            plan += [(1, 0, W), (1, W, N)]
            plan += [(t, 0, N) for t in range(2, T)]
            stq = [nc.sync] * 4 + [nc.gpsimd, nc.gpsimd, nc.scalar]
            stq += [nc.sync, nc.scalar] * 3
            for (t, c0, c1), q in zip(plan, stq):
                cols = slice(c0, c1)
                nc.vector.tensor_tensor(
                    out=tiles[t][:, cols],
                    in0=tiles[t][:, cols],
                    in1=cvec[:, cols],
                    op=mult,
                )
                q.dma_start(out=o_t[t][:, cols], in_=tiles[t][:, cols])

    nc.compile()
    return nc


def kernel(adjacency_matrix, _trace=False):
    import ml_dtypes
    from concourse.bass_utils import run_bass_kernel_spmd

    A = np.asarray(adjacency_matrix)
    assert A.shape == (N, N), A.shape
    A_bf = A.astype(ml_dtypes.bfloat16)

    if "nc" not in _cache:
        _cache["nc"] = _build()
    nc = _cache["nc"]

    in_maps = [{"a_shard": A_bf[c * R : (c + 1) * R]} for c in range(NCORES)]
    res = run_bass_kernel_spmd(
        nc, in_maps, core_ids=list(range(NCORES)), trace=_trace
    )
    _cache["last"] = res
    return np.concatenate(
        [res.results[c]["out_shard"] for c in range(NCORES)], axis=0
    ).astype(np.float32)


# revision 40
# speedup vs baseline: 1.0747x; 1.0747x over previous
# Laplacian normalization kernel for Trainium2 (8 NeuronCores, SPMD).
#
# out = d^-1/2[:, None] * A * d^-1/2[None, :],  d_i = sum_j A[i, j],  A: [8192, 8192] f32
#
# The correctness gate is rel_err < 2e-2, so the whole data path runs in
# bf16 (~1.3% end-to-end max rel err measured on the real inputs): the host
# casts A to bf16, the device reads/writes bf16, the host upcasts the
# result. That halves HBM traffic vs f32 AND lets the full 16MB row-shard
# stay resident in SBUF (128KB of ~208KB/partition), so A is read once.
#
# Per core (1024 rows = 8 tiles of [128, 8192] bf16):
#   phase A: 16 x 1MB loads alternating both HWDGE queues (~52us). Row
#   sums ride along on ACT (activation-Copy with accum_out, ~7us/tile)
#   and DVE (reduce_sum, 1x, ~9us/tile) — DVE alone would take 74us since
#   tensor_reduce has no fast mode. The last TWO tiles are split
#   half-and-half across both engines so the final row-sum lands ~4us
#   after the last load.
#   collective: d^-1/2 f32 -> PE-transpose -> bf16 [1,1024] -> AllGather
#   (2KB -> 16KB). A warmup AllGather at t=0 absorbs cc-stream setup.
#   During the collective window DVE row-scales all tiles in place
#   (tensor_scalar, 4x mode, ~19us — needs no remote data).
#   phase C: cvec ([128,8192] bf16 = gathered d^-1/2 replicated across
#   partitions) is built by a PE ones-matmul into PSUM + ACT copy-downcast
#   per 2048-col chunk instead of a 2MB broadcast DMA: keeps the
#   partition-broadcast OFF the DMA queues, which the 16MB of stores
#   saturate. Then per half-tile: tensor_tensor by cvec (2x mode) + store,
#   alternating queues.

import numpy as np

N = 8192
NCORES = 8
R = N // NCORES  # 1024 rows per core
P = 128          # SBUF partitions
T = R // P       # 8 row-tiles of [128, 8192] per core
HC = 2           # load/store chunks per tile (1MB each)
W = N // HC      # chunk width (4096 columns)
BC = 16          # cvec build chunks
BW = N // BC     # 512 columns: matmul out must fit one 2KB PSUM bank

ACT_TILES = (0, 2, 4, 6)  # full-tile row-sums on ACT; DVE takes 1, 3, 5
SPLIT_TILES = (7,)        # split ACT/DVE half-and-half (critical tail)

_cache = {}


def _build():
    import concourse.bacc as bacc
    import concourse.mybir as mybir
    import concourse.tile as tile
    from concourse import masks

    f32 = mybir.dt.float32
    bf16 = mybir.dt.bfloat16
    X = mybir.AxisListType.X
    mult = mybir.AluOpType.mult
    add = mybir.AluOpType.add
    Copy = mybir.ActivationFunctionType.Copy

    nc = bacc.Bacc(
        "TRN2", target_bir_lowering=False, debug=False, num_devices=NCORES
    )
    a = nc.dram_tensor("a_shard", [R, N], bf16, kind="ExternalInput").ap()
    out = nc.dram_tensor("out_shard", [R, N], bf16, kind="ExternalOutput").ap()

    a_t = a.rearrange("(t p) n -> t p n", p=P)
    o_t = out.rearrange("(t p) n -> t p n", p=P)

    with tile.TileContext(nc) as tc:
        with (
            tc.tile_pool(name="cpool", bufs=1) as cpool,
            tc.tile_pool(name="vpool", bufs=1) as vpool,
            tc.tile_pool(name="psum", bufs=1, space="PSUM") as psum,
            tc.tile_pool(name="bpsum", bufs=3, space="PSUM") as bpsum,
            tc.tile_pool(name="dram", bufs=1, space="DRAM") as dram,
        ):
            dsum = vpool.tile([P, T], f32, tag="dsum")
            dinv = vpool.tile([P, T], f32, tag="dinv")
            hp = vpool.tile([P, 12], f32, tag="hp")
            cvec = vpool.tile([P, N], bf16, tag="cvec")
            ident = vpool.tile([P, P], f32, tag="ident")
            ones = vpool.tile([8, P], bf16, tag="ones")
            dvec = vpool.tile([8, N], bf16, tag="dvec")
            dinv_tp = vpool.tile([T, P], bf16, tag="dinv_tp")
            dinv_tpp = psum.tile([T, P], f32, tag="dinv_tpp")
            dloc = dram.tile([1, R], bf16, tag="dloc")
            dfull = dram.tile([1, N], bf16, tag="dfull")
            warm = dram.tile([1, 8], bf16, tag="warm")
            warm_o = dram.tile([1, 8 * NCORES], bf16, tag="warm_o")

            # 3MB of loads ride the SWDGE (Pool) queue as a third DMA path
            # — dispatched before the warmup collective so the in-order
            # Pool queue cannot block them. Targets are DVE-reduced tiles,
            # which have end-of-phase slack if SWDGE turns out slower.
            tiles = []
            for t in range(T):
                big = cpool.tile([P, N], bf16, tag=f"c{t}")
                tiles.append(big)
            GP_LOADS = ((3, 0), (3, 1), (5, 0))
            for t, h in GP_LOADS:
                cols = slice(h * W, (h + 1) * W)
                nc.gpsimd.dma_start(
                    out=tiles[t][:, cols], in_=a_t[t][:, cols]
                )

            # warmup collective: absorbs cc-stream setup before the timed
            # dependency chain needs it
            nc.vector.memset(dinv_tp[0:1, 0:8], 0.0)
            nc.gpsimd.dma_start(out=warm[0, :], in_=dinv_tp[0:1, 0:8])
            nc.gpsimd.collective_compute(
                "AllGather",
                mybir.AluOpType.bypass,
                replica_groups=[list(range(NCORES))],
                ins=[warm[0, :].opt()],
                outs=[warm_o[0, :].opt()],
            )

            masks.make_identity(nc, ident[:, :])
            nc.vector.memset(ones[:, :], 1.0)

            ld = [nc.sync, nc.scalar]
            qbytes = [0, 0]

            def hw_loads(t):
                big = tiles[t]
                nch = 4 if t in (0, 1, T - 1) else HC
                w = N // nch
                for h in range(nch):
                    if (t, h * w // W) in GP_LOADS and w == W:
                        continue  # already dispatched on the Pool queue
                    cols = slice(h * w, (h + 1) * w)
                    # emptier-queue dispatch: plain alternation leaves the
                    # queues 7MB/6MB (SWDGE skips flip the parity), so the
                    # last chunks land ~3us later than they need to
                    q = 0 if qbytes[0] <= qbytes[1] else 1
                    ld[q].dma_start(out=big[:, cols], in_=a_t[t][:, cols])
                    qbytes[q] += w

            # phase A: load dispatches are software-pipelined TWO tiles
            # ahead of the reduces. nc.scalar.dma_start is dispatched by
            # the ACT engine, and the HWDGE ring holds only ~4
            # descriptors: dispatching everything up front blocks the
            # engine on ring space (measured +25us), while dispatching
            # each tile right before its own reduce starves the ring
            # behind every 7us ACT reduce (measured ~12us/tile of lag).
            # Two tiles of lookahead keeps the ring full either way.
            # Tiles 0/1 load in 2048-col chunks to fill the DMA pipe
            # faster at startup; tile 7's second half loads and reduces in
            # 2048-col quarters so the last (critical-path) reduce is
            # short.
            hw_loads(0)
            hw_loads(1)
            for t in range(T):
                big = tiles[t]
                w = N // 4
                if t + 2 < T:
                    hw_loads(t + 2)
                if t in SPLIT_TILES:
                    # ACT takes the first half; DVE reduces the last two
                    # quarters as they land, then combines
                    nc.scalar.activation(
                        out=big[:, 0:W],
                        in_=big[:, 0:W],
                        func=Copy,
                        accum_out=hp[:, 9:10],
                    )
                    nc.vector.reduce_sum(
                        out=hp[:, 10:11], in_=big[:, W : W + w], axis=X
                    )
                    nc.vector.reduce_sum(
                        out=hp[:, 11:12], in_=big[:, W + w : N], axis=X
                    )
                    nc.vector.tensor_scalar(
                        out=dsum[:, t : t + 1],
                        in0=hp[:, 10:11],
                        scalar1=hp[:, 9:10],
                        scalar2=hp[:, 11:12],
                        op0=add,
                        op1=add,
                    )
                elif t in ACT_TILES:
                    nc.scalar.activation(
                        out=big[:, :],
                        in_=big[:, :],
                        func=Copy,
                        accum_out=dsum[:, t : t + 1],
                    )
                else:
                    # DVE reduces go per half-tile: if the Tile scheduler
                    # misorders DVE ops against actual DMA arrival (seen on
                    # HW), the inversion costs <=4.4us instead of 8.7us
                    nc.vector.reduce_sum(
                        out=hp[:, t : t + 1], in_=big[:, 0:W], axis=X
                    )
                    nc.vector.reduce_sum(
                        out=hp[:, t + 1 : t + 2], in_=big[:, W:N], axis=X
                    )
                    nc.vector.tensor_tensor(
                        out=dsum[:, t : t + 1],
                        in0=hp[:, t : t + 1],
                        in1=hp[:, t + 1 : t + 2],
                        op=add,
                    )

            # d^-1/2 (ACT sqrt + DVE reciprocal; ACT Rsqrt is banned), then
            # PE-transpose [128, T] -> [T, P] so the collective input is one
            # contiguous 2KB DMA
            nc.scalar.sqrt(dsum[:, :], dsum[:, :])
            nc.vector.reciprocal(dinv[:, :], dsum[:, :])
            nc.tensor.transpose(dinv_tpp[:, :], dinv[:, :], ident[:, :])
            # PSUM->SBUF downcast on DVE: keeps ACT's second Copy-table
            # switch off the critical chain
            nc.vector.tensor_copy(out=dinv_tp[:, :], in_=dinv_tpp[:, :])
            nc.sync.dma_start(out=dloc[0, :], in_=dinv_tp[:, :])

            nc.gpsimd.collective_compute(
                "AllGather",
                mybir.AluOpType.bypass,
                replica_groups=[list(range(NCORES))],
                ins=[dloc[0, :].opt()],
                outs=[dfull[0, :].opt()],
            )

            # row scale (local, DVE 4x) — runs during the collective window
            for t in range(T):
                nc.vector.tensor_scalar(
                    out=tiles[t][:, :],
                    in0=tiles[t][:, :],
                    scalar1=dinv[:, t : t + 1],
                    scalar2=None,
                    op0=mult,
                )

            # cvec = gathered d^-1/2 replicated across partitions. The first
            # 1024 cols come from a small direct broadcast DMA (256KB) so
            # the first col-scale starts ~4us after the collective; the
            # rest is built on PE (ones[8,128].T @ dvec[8,N] = 8*d^-1/2 ->
            # PSUM) + ACT copy with scale=1/8 (exact exponent shift), which
            # keeps the bulk partition-broadcast off the DMA queues that
            # the stores saturate.
            nc.scalar.dma_start(
                out=cvec[:, 0 : 2 * BW],
                in_=dfull[0:1, 0 : 2 * BW].to_broadcast((P, 2 * BW)),
            )
            nc.sync.dma_start(
                out=dvec[:, :], in_=dfull[0:1, :].to_broadcast((8, N))
            )
            for b in range(2, BC):
                cols = slice(b * BW, (b + 1) * BW)
                pb = bpsum.tile([P, BW], f32, tag="bc")
                nc.tensor.matmul(
                    pb[:, :], ones[:, :], dvec[:, cols], start=True, stop=True
                )
                nc.scalar.activation(
                    out=cvec[:, cols], in_=pb[:, :], func=Copy, scale=0.125
                )

            # phase C: col scale (tensor_tensor, 2x) + store per half-tile.
            # Tile 0's first half goes in 1024-col slivers so the first
            # store launches as soon as cvec chunk 0 lands — the stores
            # are the phase-C bottleneck (~317 GB/s), every us of earlier
            # start is a us off the tail.
            plan = [(0, b * BW * 2, (b + 1) * BW * 2) for b in range(4)]
            plan += [(0, W, N)]
            plan += [(t, h * W, (h + 1) * W) for t in range(1, T) for h in range(HC)]
            # early stores go on the SP queue: the ACT engine (scalar
            # queue dispatcher) is busy emitting cvec copies right after
            # the collective; bytes balance to 8MB per queue overall
            # two early big stores ride SWDGE too (produced ~50us before
            # the drain ends, so even a slow third queue relieves 2MB)
            stq = [nc.sync] * 4 + [nc.gpsimd, nc.gpsimd] + [nc.scalar]
            stq += [nc.sync, nc.scalar] * 6
            for (t, c0, c1), q in zip(plan, stq):
                cols = slice(c0, c1)
                nc.vector.tensor_tensor(
                    out=tiles[t][:, cols],
                    in0=tiles[t][:, cols],
                    in1=cvec[:, cols],
                    op=mult,
                )
                q.dma_start(out=o_t[t][:, cols], in_=tiles[t][:, cols])

    nc.compile()
    return nc


def kernel(adjacency_matrix, _trace=False):
    import ml_dtypes
    from concourse.bass_utils import run_bass_kernel_spmd

    A = np.asarray(adjacency_matrix)
    assert A.shape == (N, N), A.shape
    A_bf = A.astype(ml_dtypes.bfloat16)

    if "nc" not in _cache:
        _cache["nc"] = _build()
    nc = _cache["nc"]

    in_maps = [{"a_shard": A_bf[c * R : (c + 1) * R]} for c in range(NCORES)]
    res = run_bass_kernel_spmd(
        nc, in_maps, core_ids=list(range(NCORES)), trace=_trace
    )
    _cache["last"] = res
    return np.concatenate(
        [res.results[c]["out_shard"] for c in range(NCORES)], axis=0
    ).astype(np.float32)


# revision 41
# speedup vs baseline: 1.0780x; 1.0031x over previous
# Laplacian normalization kernel for Trainium2 (8 NeuronCores, SPMD).
#
# out = d^-1/2[:, None] * A * d^-1/2[None, :],  d_i = sum_j A[i, j],  A: [8192, 8192] f32
#
# The correctness gate is rel_err < 2e-2, so the whole data path runs in
# bf16 (~1.3% end-to-end max rel err measured on the real inputs): the host
# casts A to bf16, the device reads/writes bf16, the host upcasts the
# result. That halves HBM traffic vs f32 AND lets the full 16MB row-shard
# stay resident in SBUF (128KB of ~208KB/partition), so A is read once.
#
# Per core (1024 rows = 8 tiles of [128, 8192] bf16):
#   phase A: 16 x 1MB loads alternating both HWDGE queues (~52us). Row
#   sums ride along on ACT (activation-Copy with accum_out, ~7us/tile)
#   and DVE (reduce_sum, 1x, ~9us/tile) — DVE alone would take 74us since
#   tensor_reduce has no fast mode. The last TWO tiles are split
#   half-and-half across both engines so the final row-sum lands ~4us
#   after the last load.
#   collective: d^-1/2 f32 -> PE-transpose -> bf16 [1,1024] -> AllGather
#   (2KB -> 16KB). A warmup AllGather at t=0 absorbs cc-stream setup.
#   During the collective window DVE row-scales all tiles in place
#   (tensor_scalar, 4x mode, ~19us — needs no remote data).
#   phase C: cvec ([128,8192] bf16 = gathered d^-1/2 replicated across
#   partitions) is built by a PE ones-matmul into PSUM + ACT copy-downcast
#   per 2048-col chunk instead of a 2MB broadcast DMA: keeps the
#   partition-broadcast OFF the DMA queues, which the 16MB of stores
#   saturate. Then per half-tile: tensor_tensor by cvec (2x mode) + store,
#   alternating queues.

import numpy as np

N = 8192
NCORES = 8
R = N // NCORES  # 1024 rows per core
P = 128          # SBUF partitions
T = R // P       # 8 row-tiles of [128, 8192] per core
HC = 2           # load/store chunks per tile (1MB each)
W = N // HC      # chunk width (4096 columns)
BC = 16          # cvec build chunks
BW = N // BC     # 512 columns: matmul out must fit one 2KB PSUM bank

ACT_TILES = (0, 2, 4, 6)  # full-tile row-sums on ACT; DVE takes 1, 3, 5
SPLIT_TILES = (7,)        # split ACT/DVE half-and-half (critical tail)

_cache = {}


def _build():
    import concourse.bacc as bacc
    import concourse.mybir as mybir
    import concourse.tile as tile
    from concourse import masks

    f32 = mybir.dt.float32
    bf16 = mybir.dt.bfloat16
    X = mybir.AxisListType.X
    mult = mybir.AluOpType.mult
    add = mybir.AluOpType.add
    Copy = mybir.ActivationFunctionType.Copy

    nc = bacc.Bacc(
        "TRN2", target_bir_lowering=False, debug=False, num_devices=NCORES
    )
    a = nc.dram_tensor("a_shard", [R, N], bf16, kind="ExternalInput").ap()
    out = nc.dram_tensor("out_shard", [R, N], bf16, kind="ExternalOutput").ap()

    a_t = a.rearrange("(t p) n -> t p n", p=P)
    o_t = out.rearrange("(t p) n -> t p n", p=P)

    with tile.TileContext(nc) as tc:
        with (
            tc.tile_pool(name="cpool", bufs=1) as cpool,
            tc.tile_pool(name="vpool", bufs=1) as vpool,
            tc.tile_pool(name="psum", bufs=1, space="PSUM") as psum,
            tc.tile_pool(name="bpsum", bufs=3, space="PSUM") as bpsum,
            tc.tile_pool(name="dram", bufs=1, space="DRAM") as dram,
        ):
            dsum = vpool.tile([P, T], f32, tag="dsum")
            dinv = vpool.tile([P, T], f32, tag="dinv")
            hp = vpool.tile([P, 12], f32, tag="hp")
            cvec = vpool.tile([P, N], bf16, tag="cvec")
            ident = vpool.tile([P, P], f32, tag="ident")
            ones = vpool.tile([8, P], bf16, tag="ones")
            dvec = vpool.tile([8, N], bf16, tag="dvec")
            dinv_tp = vpool.tile([T, P], bf16, tag="dinv_tp")
            dinv_tpp = psum.tile([T, P], f32, tag="dinv_tpp")
            dloc = dram.tile([1, R], bf16, tag="dloc")
            dfull = dram.tile([1, N], bf16, tag="dfull")
            warm = dram.tile([1, 8], bf16, tag="warm")
            warm_o = dram.tile([1, 8 * NCORES], bf16, tag="warm_o")

            # 3MB of loads ride the SWDGE (Pool) queue as a third DMA path
            # — dispatched before the warmup collective so the in-order
            # Pool queue cannot block them. Targets are DVE-reduced tiles,
            # which have end-of-phase slack if SWDGE turns out slower.
            tiles = []
            for t in range(T):
                big = cpool.tile([P, N], bf16, tag=f"c{t}")
                tiles.append(big)
            GP_LOADS = ((3, 0), (3, 1), (5, 0))
            for t, h in GP_LOADS:
                cols = slice(h * W, (h + 1) * W)
                nc.gpsimd.dma_start(
                    out=tiles[t][:, cols], in_=a_t[t][:, cols]
                )

            # warmup collective: absorbs cc-stream setup before the timed
            # dependency chain needs it
            nc.vector.memset(dinv_tp[0:1, 0:8], 0.0)
            nc.gpsimd.dma_start(out=warm[0, :], in_=dinv_tp[0:1, 0:8])
            nc.gpsimd.collective_compute(
                "AllGather",
                mybir.AluOpType.bypass,
                replica_groups=[list(range(NCORES))],
                ins=[warm[0, :].opt()],
                outs=[warm_o[0, :].opt()],
            )

            masks.make_identity(nc, ident[:, :])
            nc.vector.memset(ones[:, :], 1.0)

            ld = [nc.sync, nc.scalar]
            nld = [0]

            def hw_loads(t):
                big = tiles[t]
                nch = 4 if t in (0, 1, T - 1) else HC
                w = N // nch
                for h in range(nch):
                    if (t, h * w // W) in GP_LOADS and w == W:
                        continue  # already dispatched on the Pool queue
                    cols = slice(h * w, (h + 1) * w)
                    ld[nld[0] % 2].dma_start(
                        out=big[:, cols], in_=a_t[t][:, cols]
                    )
                    nld[0] += 1

            # phase A: load dispatches are software-pipelined TWO tiles
            # ahead of the reduces. nc.scalar.dma_start is dispatched by
            # the ACT engine, and the HWDGE ring holds only ~4
            # descriptors: dispatching everything up front blocks the
            # engine on ring space (measured +25us), while dispatching
            # each tile right before its own reduce starves the ring
            # behind every 7us ACT reduce (measured ~12us/tile of lag).
            # Two tiles of lookahead keeps the ring full either way.
            # Tiles 0/1 load in 2048-col chunks to fill the DMA pipe
            # faster at startup; tile 7's second half loads and reduces in
            # 2048-col quarters so the last (critical-path) reduce is
            # short.
            hw_loads(0)
            hw_loads(1)
            for t in range(T):
                big = tiles[t]
                w = N // 4
                if t + 2 < T:
                    hw_loads(t + 2)
                if t in SPLIT_TILES:
                    # ACT takes the first half; DVE reduces the last two
                    # quarters as they land, then combines
                    nc.scalar.activation(
                        out=big[:, 0:W],
                        in_=big[:, 0:W],
                        func=Copy,
                        accum_out=hp[:, 9:10],
                    )
                    nc.vector.reduce_sum(
                        out=hp[:, 10:11], in_=big[:, W : W + w], axis=X
                    )
                    nc.vector.reduce_sum(
                        out=hp[:, 11:12], in_=big[:, W + w : N], axis=X
                    )
                    nc.vector.tensor_scalar(
                        out=dsum[:, t : t + 1],
                        in0=hp[:, 10:11],
                        scalar1=hp[:, 9:10],
                        scalar2=hp[:, 11:12],
                        op0=add,
                        op1=add,
                    )
                elif t in ACT_TILES:
                    nc.scalar.activation(
                        out=big[:, :],
                        in_=big[:, :],
                        func=Copy,
                        accum_out=dsum[:, t : t + 1],
                    )
                else:
                    # DVE reduces go per half-tile: if the Tile scheduler
                    # misorders DVE ops against actual DMA arrival (seen on
                    # HW), the inversion costs <=4.4us instead of 8.7us
                    nc.vector.reduce_sum(
                        out=hp[:, t : t + 1], in_=big[:, 0:W], axis=X
                    )
                    nc.vector.reduce_sum(
                        out=hp[:, t + 1 : t + 2], in_=big[:, W:N], axis=X
                    )
                    nc.vector.tensor_tensor(
                        out=dsum[:, t : t + 1],
                        in0=hp[:, t : t + 1],
                        in1=hp[:, t + 1 : t + 2],
                        op=add,
                    )

            # d^-1/2 (ACT sqrt + DVE reciprocal; ACT Rsqrt is banned), then
            # PE-transpose [128, T] -> [T, P] so the collective input is one
            # contiguous 2KB DMA
            nc.scalar.sqrt(dsum[:, :], dsum[:, :])
            nc.vector.reciprocal(dinv[:, :], dsum[:, :])
            nc.tensor.transpose(dinv_tpp[:, :], dinv[:, :], ident[:, :])
            # PSUM->SBUF downcast on DVE: keeps ACT's second Copy-table
            # switch off the critical chain
            nc.vector.tensor_copy(out=dinv_tp[:, :], in_=dinv_tpp[:, :])
            nc.sync.dma_start(out=dloc[0, :], in_=dinv_tp[:, :])

            nc.gpsimd.collective_compute(
                "AllGather",
                mybir.AluOpType.bypass,
                replica_groups=[list(range(NCORES))],
                ins=[dloc[0, :].opt()],
                outs=[dfull[0, :].opt()],
            )

            # row scale (local, DVE 4x) — runs during the collective window
            for t in range(T):
                nc.vector.tensor_scalar(
                    out=tiles[t][:, :],
                    in0=tiles[t][:, :],
                    scalar1=dinv[:, t : t + 1],
                    scalar2=None,
                    op0=mult,
                )

            # cvec = gathered d^-1/2 replicated across partitions. The first
            # 1024 cols come from a small direct broadcast DMA (256KB) so
            # the first col-scale starts ~4us after the collective; the
            # rest is built on PE (ones[8,128].T @ dvec[8,N] = 8*d^-1/2 ->
            # PSUM) + ACT copy with scale=1/8 (exact exponent shift), which
            # keeps the bulk partition-broadcast off the DMA queues that
            # the stores saturate.
            nc.scalar.dma_start(
                out=cvec[:, 0 : 2 * BW],
                in_=dfull[0:1, 0 : 2 * BW].to_broadcast((P, 2 * BW)),
            )
            nc.sync.dma_start(
                out=dvec[:, :], in_=dfull[0:1, :].to_broadcast((8, N))
            )
            for b in range(2, BC):
                cols = slice(b * BW, (b + 1) * BW)
                pb = bpsum.tile([P, BW], f32, tag="bc")
                nc.tensor.matmul(
                    pb[:, :], ones[:, :], dvec[:, cols], start=True, stop=True
                )
                nc.scalar.activation(
                    out=cvec[:, cols], in_=pb[:, :], func=Copy, scale=0.125
                )

            # phase C: col scale (tensor_tensor, 2x) + store per half-tile.
            # Tile 0's first half goes in 1024-col slivers so the first
            # store launches as soon as cvec chunk 0 lands — the stores
            # are the phase-C bottleneck (~317 GB/s), every us of earlier
            # start is a us off the tail.
            plan = [(0, b * BW * 2, (b + 1) * BW * 2) for b in range(4)]
            plan += [(0, W, N)]
            plan += [(t, h * W, (h + 1) * W) for t in range(1, T) for h in range(HC)]
            # early stores go on the SP queue: the ACT engine (scalar
            # queue dispatcher) is busy emitting cvec copies right after
            # the collective; bytes balance to 8MB per queue overall
            # two early big stores ride SWDGE too (produced ~50us before
            # the drain ends, so even a slow third queue relieves 2MB)
            stq = [nc.sync] * 4 + [nc.gpsimd, nc.gpsimd] + [nc.scalar]
            stq += [nc.sync, nc.scalar] * 6
            for (t, c0, c1), q in zip(plan, stq):
                cols = slice(c0, c1)
                nc.vector.tensor_tensor(
                    out=tiles[t][:, cols],
                    in0=tiles[t][:, cols],
                    in1=cvec[:, cols],
                    op=mult,
                )
                q.dma_start(out=o_t[t][:, cols], in_=tiles[t][:, cols])

    nc.compile()
    return nc


def kernel(adjacency_matrix, _trace=False):
    import ml_dtypes
    from concourse.bass_utils import run_bass_kernel_spmd

    A = np.asarray(adjacency_matrix)
    assert A.shape == (N, N), A.shape
    A_bf = A.astype(ml_dtypes.bfloat16)

    if "nc" not in _cache:
        _cache["nc"] = _build()
    nc = _cache["nc"]

    in_maps = [{"a_shard": A_bf[c * R : (c + 1) * R]} for c in range(NCORES)]
    res = run_bass_kernel_spmd(
        nc, in_maps, core_ids=list(range(NCORES)), trace=_trace
    )
    _cache["last"] = res
    return np.concatenate(
        [res.results[c]["out_shard"] for c in range(NCORES)], axis=0
    ).astype(np.float32)
